# revision 30
# baseline (speedup 1.0000x reference)
"""AttentionConv1d Trainium2 kernel — 8-core batch-parallel SPMD.

Reference semantics (B=8, C=512, T=4096, O=512, K=3):
    out[b,o,t] = sum_{c,k} feature[b,c,t+k-1] * sim[b, (3c+k)//512, t] * weight[o,c,k]
where sim[b,0/1/2,t] are cosine similarities of embedding column t with its
left neighbor / itself / right neighbor (zero-padded at the edges), so
sim[:,1,:] == 1 for any column with norm >= eps.

Decomposition per batch element (one NeuronCore each):
    conv_j[o,t] = sum over the (c,k) pairs with (3c+k)//512 == j
    out = sim_l * conv_0 + conv_1 + sim_r * conv_2
computed transposed (out_T[t,o]) so sim_l/sim_r become per-partition scalars.
PE chunks are 32-aligned in the partition dim (tile_position constraint);
j-boundary straddles are handled with host-side zero-masked weight copies.
The conv runs in bf16 (host-cast), PSUM accumulation in fp32.
"""
from contextlib import ExitStack

import ml_dtypes
import numpy as np

import concourse.bass as bass
import concourse.tile as tile
from concourse import mybir
from concourse.bass_utils import run_bass_kernel_spmd

F32 = mybir.dt.float32
BF16 = mybir.dt.bfloat16

B, C, T, O, K = 8, 512, 4096, 512, 3
CP = C // 128  # 4 c-tiles
TQ = T // 128  # 32 t-tiles
NB = T // 512  # 8 reduce blocks


def _j_bounds(k):
    """(b0, b1): first c with (3c+k) >= 512 / >= 1024."""
    b0 = (512 - k + 2) // 3
    b1 = (1024 - k + 2) // 3
    return b0, b1


# custom F row-group tiles: slot -> (c_start, rows)
_F_SLOTS = [(0, 128), (128, 64), (160, 128), (288, 64), (320, 64), (384, 128)]
_BIG = {0: 0, 1: 2, 2: 5}  # j -> big slot (128 rows)
_SMALL = {0: 1, 1: 3, 2: 4}  # j -> small slot (64 rows)
# per j: 3 big chunks (k=0,1,2), 1 combo chunk (small rows k=0 stacked on
# small rows pre-shifted for k=1), 1 small chunk (k=2). 15 matmuls per
# out-tile. Out-of-j rows are zero-masked in the packed weights.
_N_SLOT_ROWS = sum(r for _, r in _F_SLOTS)  # 576
_N_F_ROWS = _N_SLOT_ROWS + 3 * 128  # + combo tiles
_W_ROWS_PER_J = 3 * 128 + 128 + 64  # 576


def host_prep(feature, embedding, weight):
    """Build per-core input maps: padded F/E shards + packed masked weights."""
    feature = np.ascontiguousarray(np.asarray(feature, dtype=np.float32))
    embedding = np.ascontiguousarray(np.asarray(embedding, dtype=np.float32))
    weight = np.ascontiguousarray(np.asarray(weight, dtype=np.float32))

    f_pad = np.pad(feature, ((0, 0), (0, 0), (1, 1)))  # [B, C, T+2]
    f_parts = [f_pad[:, c0 : c0 + rows, :] for c0, rows in _F_SLOTS]
    for j in range(3):  # combo tiles: small rows as-is over small rows shifted +1
        c0, rows = _F_SLOTS[_SMALL[j]]
        sl = f_pad[:, c0 : c0 + rows, :]
        sh = np.concatenate([sl[:, :, 1:], np.zeros_like(sl[:, :, :1])], axis=2)
        f_parts.append(np.concatenate([sl, sh], axis=1))
    f_slotted = np.concatenate(f_parts, axis=1).astype(
        ml_dtypes.bfloat16
    )  # [B, 960, T+2]
    e_pad = np.pad(embedding, ((0, 0), (0, 0), (1, 0)))  # [B, C, T+1] f32

    def wmask(k, j, c):
        b0, b1 = _j_bounds(k)
        lo, hi = [(0, b0), (b0, b1), (b1, C)][j]
        return weight[:, c, k] if lo <= c < hi else np.zeros(O, np.float32)

    w_parts = []
    for j in range(3):
        cb = _F_SLOTS[_BIG[j]][0]
        cs = _F_SLOTS[_SMALL[j]][0]
        for k in range(K):  # big chunks
            w_parts.append(np.stack([wmask(k, j, cb + r) for r in range(128)]))
        w_parts.append(  # combo chunk: k=0 rows then k=1 rows
            np.stack(
                [wmask(0, j, cs + r) for r in range(64)]
                + [wmask(1, j, cs + r) for r in range(64)]
            )
        )
        w_parts.append(np.stack([wmask(2, j, cs + r) for r in range(64)]))  # small k=2
    w_packed = np.concatenate(w_parts).astype(ml_dtypes.bfloat16)  # [3*576, O]
    assert w_packed.shape[0] == 3 * _W_ROWS_PER_J
    in_maps = [
        {
            "feature": np.ascontiguousarray(f_slotted[b]),
            "embedding": np.ascontiguousarray(e_pad[b]),
            "weight_t": w_packed,
        }
        for b in range(B)
    ]
    return in_maps


def _fix_sync_waits(nc, limit=1):
    """Split instructions with more sem waits than walrus' TPB encoding allows."""
    counter = 0
    for f in nc.m.functions:
        for bb in f.blocks:
            insts = list(bb.instructions)
            new_insts = []
            changed = False
            for inst in insts:
                si = inst.sync_info
                waits = list(si.on_wait) if si and si.on_wait else []
                if len(waits) > limit:
                    changed = True
                    head, rest = waits[:-limit], waits[-limit:]
                    for i in range(0, len(head), limit):
                        counter += 1
                        nop = mybir.InstNoOp(name=f"I-waitsplit-{counter}")
                        nop.engine = inst.engine
                        nop.sync_info = mybir.SyncInfo(
                            on_wait=head[i : i + limit], on_update=[]
                        )
                        new_insts.append(nop)
                    inst.sync_info = mybir.SyncInfo(
                        on_wait=rest, on_update=list(si.on_update or [])
                    )
                new_insts.append(inst)
            if changed:
                bb.instructions.clear()
                for i in new_insts:
                    bb.add_instruction(i)
    return counter


def build_kernel():
    nc = bass.Bass(target_bir_lowering=False, trn_type="TRN2")
    Fd = nc.declare_dram_parameter("feature", [_N_F_ROWS, T + 2], BF16, isOutput=False)
    Ed = nc.declare_dram_parameter("embedding", [C, T + 1], F32, isOutput=False)
    Wd = nc.declare_dram_parameter(
        "weight_t", [3 * _W_ROWS_PER_J, O], BF16, isOutput=False
    )
    Od = nc.declare_dram_parameter("out", [T, O], F32, isOutput=True)

    with tile.TileContext(nc) as tc, ExitStack() as ctx:
        body(ctx, tc, Fd, Ed, Wd, Od)
    _fix_sync_waits(nc, limit=1)
    return nc


def body(ctx, tc, Fd, Ed, Wd, Od):
    nc = tc.nc

    consts = ctx.enter_context(tc.tile_pool(name="consts", bufs=1))
    fpool = ctx.enter_context(tc.tile_pool(name="fpool", bufs=1))
    wpool = ctx.enter_context(tc.tile_pool(name="wpool", bufs=1))
    epool = ctx.enter_context(tc.tile_pool(name="epool", bufs=4))
    sqpool = ctx.enter_context(tc.tile_pool(name="sqpool", bufs=4))
    rowpool = ctx.enter_context(tc.tile_pool(name="rowpool", bufs=1))
    simpool = ctx.enter_context(tc.tile_pool(name="simpool", bufs=1))
    outpool = ctx.enter_context(tc.tile_pool(name="outpool", bufs=4))

    # --- constants ---
    ones_t = consts.tile([128, 128], BF16, tag="ones")
    nc.vector.memset(ones_t[:], 1.0)
    e0 = consts.tile([128, 1], BF16, tag="e0")
    nc.vector.memset(e0[:], 0.0)
    nc.vector.memset(e0[0:1, :], 1.0)

    # --- E slices first: the reduce phase consumes them before F/W are needed
    e_slices = {}
    for b in range(NB // 2):
        for p in range(CP):
            esl = epool.tile([128, 1025], F32, tag="esl", name=f"esl{b}_{p}")
            nc.sync.dma_start(
                esl[:], Ed[128 * p : 128 * p + 128, 1024 * b : 1024 * b + 1025]
            )
            e_slices[(b, p)] = esl

    # --- resident inputs: F slot + combo tiles (chunks start at partition 0)
    f_tiles = []
    row = 0
    for s, (_, rows) in enumerate(_F_SLOTS):
        ft = fpool.tile([rows, T + 2], BF16, tag=f"f{s}", name=f"f{s}")
        nc.sync.dma_start(ft[:], Fd[row : row + rows, :])
        f_tiles.append(ft)
        row += rows
    f_combo = []
    for j in range(3):
        ft = fpool.tile([128, T + 2], BF16, tag=f"fc{j}", name=f"fc{j}")
        nc.sync.dma_start(ft[:], Fd[row : row + 128, :])
        f_combo.append(ft)
        row += 128
    # per-j chunk list: (f_tile, rows, col_off) in accumulation order
    chunk_rows = [128, 128, 128, 128, 64]
    w_tiles = {}
    row = 0
    for j in range(3):
        for ci, rows in enumerate(chunk_rows):
            wt = wpool.tile([rows, O], BF16, tag=f"w{j}_{ci}", name=f"w{j}_{ci}")
            nc.sync.dma_start(wt[:], Wd[row : row + rows, :])
            w_tiles[(j, ci)] = wt
            row += rows

    # --- sim row space: sq/pl (bf16) -> ones-matmul partition reduce -> n/dl rows ---
    n_sb = rowpool.tile([128, T + 2], BF16, tag="n_sb")
    dl_sb = rowpool.tile([128, T + 2], BF16, tag="dl_sb")
    for sb in (n_sb, dl_sb):
        nc.vector.memset(sb[:, 0:1], 0.0)
        nc.vector.memset(sb[:, T + 1 : T + 2], 0.0)

    # window-extraction variants; n_sb col 1+t = n[t]; dl_sb col 1+t = dl[t].
    variants = [
        ("nT0", n_sb, 1),
        ("nTm", n_sb, 0),
        ("nTp", n_sb, 2),
        ("dT0", dl_sb, 1),
        ("dTp", dl_sb, 2),
    ]
    with tc.tile_pool(name="redpsum", bufs=1, space="PSUM") as redpsum:
        xt_all = redpsum.tile([128, 5 * TQ], F32, tag="xt", name="xt_all")
        for b in range(NB // 2):
            s2b = redpsum.tile([128, 1024], F32, tag="s2b", name=f"s2b{b}")
            dlb = redpsum.tile([128, 1024], F32, tag="dlb", name=f"dlb{b}")
            for p in range(CP):
                esl = e_slices[(b, p)]
                sq = sqpool.tile([128, 1024], BF16, tag="sq", name=f"sq{b}_{p}")
                pl = sqpool.tile([128, 1024], BF16, tag="pl", name=f"pl{b}_{p}")
                nc.scalar.square(sq[:], esl[:, 1:1025])
                nc.vector.tensor_mul(pl[:], esl[:, 1:1025], esl[:, 0:1024])
                for h in range(2):  # matmul out limited to one PSUM bank (512 f32)
                    hs = slice(512 * h, 512 * h + 512)
                    nc.tensor.matmul(
                        s2b[:, hs], ones_t[:], sq[:, hs],
                        start=(p == 0), stop=(p == CP - 1),
                    )
                    nc.tensor.matmul(
                        dlb[:, hs], ones_t[:], pl[:, hs],
                        start=(p == 0), stop=(p == CP - 1),
                    )
            # evacuate: n = sqrt(s2) via ACT, dl plain copy via DVE (bf16 out)
            nc.scalar.sqrt(n_sb[:, 1 + 1024 * b : 1025 + 1024 * b], s2b[:])
            nc.vector.tensor_copy(dl_sb[:, 1 + 1024 * b : 1025 + 1024 * b], dlb[:])
            # interleave column extraction for the t-tiles this block completes:
            # window q needs src cols < 130+128q, written through col 1025+1024b.
            q_lo = 0 if b == 0 else (895 + 1024 * (b - 1)) // 128 + 1
            q_hi = TQ if b == NB // 2 - 1 else (895 + 1024 * b) // 128 + 1
            for q in range(q_lo, q_hi):
                for v, (name, src, off) in enumerate(variants):
                    nc.tensor.matmul(
                        xt_all[:, 32 * v + q : 32 * v + q + 1],
                        src[:, off + 128 * q : off + 128 * q + 128],
                        e0[:],
                        start=True,
                        stop=True,
                    )
        xt_sb = simpool.tile([128, 5 * TQ], F32, tag="xt_sb", name="xt_sb")
        nc.vector.tensor_copy(xt_sb[:], xt_all[:])
    cols = {
        name: xt_sb[:, 32 * v : 32 * v + 32]
        for v, (name, _, _) in enumerate(variants)
    }

    # sims: sim_l = dT0 / max(nT0*nTm, tiny); sim_r = dTp / max(nT0*nTp, tiny)
    sim_l = simpool.tile([128, TQ], F32, tag="sim_l")
    sim_r = simpool.tile([128, TQ], F32, tag="sim_r")
    for sim, nx, dx in ((sim_l, "nTm", "dT0"), (sim_r, "nTp", "dTp")):
        prod = simpool.tile([128, TQ], F32, tag=f"prod_{nx}", name=f"prod_{nx}")
        nc.vector.tensor_mul(prod[:], cols["nT0"], cols[nx])
        nc.vector.tensor_scalar_max(prod[:], prod[:], 1e-30)
        nc.vector.reciprocal(prod[:], prod[:])
        nc.vector.tensor_mul(sim[:], cols[dx], prod[:])

    # --- main conv: out_T[t,o] accumulated per 128-t tile, 15 chunks each ---
    per_j = [[], [], []]
    for j in range(3):
        for k in range(K):  # big chunks: shift = k
            per_j[j].append((f_tiles[_BIG[j]], 128, k, w_tiles[(j, k)]))
        per_j[j].append((f_combo[j], 128, 0, w_tiles[(j, 3)]))  # combo (k0|k1)
        per_j[j].append((f_tiles[_SMALL[j]], 64, 2, w_tiles[(j, 4)]))  # small k2

    convpsum = ctx.enter_context(tc.tile_pool(name="convpsum", bufs=2, space="PSUM"))
    for q in range(TQ):
        psums = [
            convpsum.tile([128, O], F32, tag=f"P{j}", name=f"P{j}_{q}")
            for j in range(3)
        ]
        for j in range(3):
            cl = per_j[j]
            for idx, (ft, rows, off, wt) in enumerate(cl):
                nc.tensor.matmul(
                    psums[j][:],
                    ft[0:rows, off + 128 * q : off + 128 * q + 128],
                    wt[0:rows, :],
                    start=(idx == 0),
                    stop=(idx == len(cl) - 1),
                )
        # epilogue (one PSUM read per instruction):
        #   ACT: tmp  = P0 * sim_l          (PSUM -> SBUF, per-partition scale)
        #   DVE: tmp2 = (P2 * sim_r) + tmp  (PSUM + SBUF)
        #   DVE: osb  = P1 + tmp2           (PSUM + SBUF)
        tmp = outpool.tile([128, O], F32, tag="tmp", name=f"tmp{q}")
        nc.scalar.mul(tmp[:], psums[0][:], sim_l[:, q : q + 1])
        tmp2 = outpool.tile([128, O], F32, tag="tmp2", name=f"tmp2_{q}")
        nc.vector.scalar_tensor_tensor(
            tmp2[:],
            psums[2][:],
            sim_r[:, q : q + 1],
            tmp[:],
            op0=mybir.AluOpType.mult,
            op1=mybir.AluOpType.add,
        )
        osb = outpool.tile([128, O], F32, tag="osb", name=f"osb{q}")
        nc.vector.tensor_add(osb[:], psums[1][:], tmp2[:])
        nc.sync.dma_start(Od[128 * q : 128 * q + 128, :], osb[:])


_NC_CACHE = {}


def _get_nc():
    if "nc" not in _NC_CACHE:
        _NC_CACHE["nc"] = build_kernel()
    return _NC_CACHE["nc"]


def kernel(feature, embedding, weight):
    in_maps = host_prep(feature, embedding, weight)
    nc = _get_nc()
    res = run_bass_kernel_spmd(nc, in_maps, core_ids=list(range(B)))
    out = np.stack([res.results[b]["out"].T for b in range(B)])  # [B, O, T]
    return np.ascontiguousarray(out)


# revision 31
# speedup vs baseline: 1.1579x; 1.1579x over previous
"""AttentionConv1d Trainium2 kernel — 8-core batch-parallel SPMD.

Reference semantics (B=8, C=512, T=4096, O=512, K=3):
    out[b,o,t] = sum_{c,k} feature[b,c,t+k-1] * sim[b, (3c+k)//512, t] * weight[o,c,k]
where sim[b,0/1/2,t] are cosine similarities of embedding column t with its
left neighbor / itself / right neighbor (zero-padded at the edges), so
sim[:,1,:] == 1 for any column with norm >= eps.

Decomposition per batch element (one NeuronCore each):
    conv_j[o,t] = sum over the (c,k) pairs with (3c+k)//512 == j
    out = sim_l * conv_0 + conv_1 + sim_r * conv_2
computed transposed (out_T[t,o]) so sim_l/sim_r become per-partition scalars.
PE chunks are 32-aligned in the partition dim (tile_position constraint);
j-boundary straddles are handled with host-side zero-masked weight copies.
The conv runs in bf16 (host-cast), PSUM accumulation in fp32.
"""
from contextlib import ExitStack

import ml_dtypes
import numpy as np

import concourse.bass as bass
import concourse.tile as tile
from concourse import mybir
from concourse.bass_utils import run_bass_kernel_spmd

F32 = mybir.dt.float32
BF16 = mybir.dt.bfloat16

B, C, T, O, K = 8, 512, 4096, 512, 3
CP = C // 128  # 4 c-tiles
TQ = T // 128  # 32 t-tiles
NB = T // 512  # 8 reduce blocks


def _j_bounds(k):
    """(b0, b1): first c with (3c+k) >= 512 / >= 1024."""
    b0 = (512 - k + 2) // 3
    b1 = (1024 - k + 2) // 3
    return b0, b1


# custom F row-group tiles: slot -> (c_start, rows)
_F_SLOTS = [(0, 128), (128, 64), (160, 128), (288, 64), (320, 64), (384, 128)]
_BIG = {0: 0, 1: 2, 2: 5}  # j -> big slot (128 rows)
_SMALL = {0: 1, 1: 3, 2: 4}  # j -> small slot (64 rows)
# per j: 3 big chunks (k=0,1,2), 1 combo chunk (small rows k=0 stacked on
# small rows pre-shifted for k=1), 1 small chunk (k=2). 15 matmuls per
# out-tile. Out-of-j rows are zero-masked in the packed weights.
_N_SLOT_ROWS = sum(r for _, r in _F_SLOTS)  # 576
_N_F_ROWS = _N_SLOT_ROWS + 3 * 128  # + combo tiles
_W_ROWS_PER_J = 3 * 128 + 128 + 64  # 576


def host_prep(feature, embedding, weight):
    """Build per-core input maps: padded F/E shards + packed masked weights."""
    feature = np.ascontiguousarray(np.asarray(feature, dtype=np.float32))
    embedding = np.ascontiguousarray(np.asarray(embedding, dtype=np.float32))
    weight = np.ascontiguousarray(np.asarray(weight, dtype=np.float32))

    f_pad = np.pad(feature, ((0, 0), (0, 0), (1, 1)))  # [B, C, T+2]
    f_parts = [f_pad[:, c0 : c0 + rows, :] for c0, rows in _F_SLOTS]
    for j in range(3):  # combo tiles: small rows as-is over small rows shifted +1
        c0, rows = _F_SLOTS[_SMALL[j]]
        sl = f_pad[:, c0 : c0 + rows, :]
        sh = np.concatenate([sl[:, :, 1:], np.zeros_like(sl[:, :, :1])], axis=2)
        f_parts.append(np.concatenate([sl, sh], axis=1))
    f_slotted = np.concatenate(f_parts, axis=1).astype(
        ml_dtypes.bfloat16
    )  # [B, 960, T+2]
    e_pad = np.pad(embedding, ((0, 0), (0, 0), (1, 0)))  # [B, C, T+1] f32

    def wmask(k, j, c):
        b0, b1 = _j_bounds(k)
        lo, hi = [(0, b0), (b0, b1), (b1, C)][j]
        return weight[:, c, k] if lo <= c < hi else np.zeros(O, np.float32)

    w_parts = []
    for j in range(3):
        cb = _F_SLOTS[_BIG[j]][0]
        cs = _F_SLOTS[_SMALL[j]][0]
        for k in range(K):  # big chunks
            w_parts.append(np.stack([wmask(k, j, cb + r) for r in range(128)]))
        w_parts.append(  # combo chunk: k=0 rows then k=1 rows
            np.stack(
                [wmask(0, j, cs + r) for r in range(64)]
                + [wmask(1, j, cs + r) for r in range(64)]
            )
        )
        w_parts.append(np.stack([wmask(2, j, cs + r) for r in range(64)]))  # small k=2
    w_packed = np.concatenate(w_parts).astype(ml_dtypes.bfloat16)  # [3*576, O]
    assert w_packed.shape[0] == 3 * _W_ROWS_PER_J
    in_maps = [
        {
            "feature": np.ascontiguousarray(f_slotted[b]),
            "embedding": np.ascontiguousarray(e_pad[b]),
            "weight_t": w_packed,
        }
        for b in range(B)
    ]
    return in_maps


def _fix_sync_waits(nc, limit=1):
    """Split instructions with more sem waits than walrus' TPB encoding allows."""
    counter = 0
    for f in nc.m.functions:
        for bb in f.blocks:
            insts = list(bb.instructions)
            new_insts = []
            changed = False
            for inst in insts:
                si = inst.sync_info
                waits = list(si.on_wait) if si and si.on_wait else []
                if len(waits) > limit:
                    changed = True
                    head, rest = waits[:-limit], waits[-limit:]
                    for i in range(0, len(head), limit):
                        counter += 1
                        nop = mybir.InstNoOp(name=f"I-waitsplit-{counter}")
                        nop.engine = inst.engine
                        nop.sync_info = mybir.SyncInfo(
                            on_wait=head[i : i + limit], on_update=[]
                        )
                        new_insts.append(nop)
                    inst.sync_info = mybir.SyncInfo(
                        on_wait=rest, on_update=list(si.on_update or [])
                    )
                new_insts.append(inst)
            if changed:
                bb.instructions.clear()
                for i in new_insts:
                    bb.add_instruction(i)
    return counter


def build_kernel():
    nc = bass.Bass(target_bir_lowering=False, trn_type="TRN2")
    Fd = nc.declare_dram_parameter("feature", [_N_F_ROWS, T + 2], BF16, isOutput=False)
    Ed = nc.declare_dram_parameter("embedding", [C, T + 1], F32, isOutput=False)
    Wd = nc.declare_dram_parameter(
        "weight_t", [3 * _W_ROWS_PER_J, O], BF16, isOutput=False
    )
    Od = nc.declare_dram_parameter("out", [T, O], F32, isOutput=True)

    with tile.TileContext(nc) as tc, ExitStack() as ctx:
        body(ctx, tc, Fd, Ed, Wd, Od)
    _fix_sync_waits(nc, limit=1)
    return nc


def body(ctx, tc, Fd, Ed, Wd, Od):
    nc = tc.nc

    consts = ctx.enter_context(tc.tile_pool(name="consts", bufs=1))
    fpool = ctx.enter_context(tc.tile_pool(name="fpool", bufs=1))
    wpool = ctx.enter_context(tc.tile_pool(name="wpool", bufs=1))
    epool = ctx.enter_context(tc.tile_pool(name="epool", bufs=4))
    sqpool = ctx.enter_context(tc.tile_pool(name="sqpool", bufs=4))
    rowpool = ctx.enter_context(tc.tile_pool(name="rowpool", bufs=1))
    simpool = ctx.enter_context(tc.tile_pool(name="simpool", bufs=1))
    outpool = ctx.enter_context(tc.tile_pool(name="outpool", bufs=4))

    # --- constants ---
    ones_t = consts.tile([128, 128], BF16, tag="ones")
    nc.vector.memset(ones_t[:], 1.0)
    e0 = consts.tile([128, 1], BF16, tag="e0")
    nc.vector.memset(e0[:], 0.0)
    nc.vector.memset(e0[0:1, :], 1.0)

    # --- E slices first: the reduce phase consumes them before F/W are needed
    e_slices = {}
    for b in range(NB // 2):
        for p in range(CP):
            esl = epool.tile([128, 1025], F32, tag="esl", name=f"esl{b}_{p}")
            nc.sync.dma_start(
                esl[:], Ed[128 * p : 128 * p + 128, 1024 * b : 1024 * b + 1025]
            )
            e_slices[(b, p)] = esl

    # --- resident inputs: F slot + combo tiles (chunks start at partition 0)
    f_tiles = []
    row = 0
    for s, (_, rows) in enumerate(_F_SLOTS):
        ft = fpool.tile([rows, T + 2], BF16, tag=f"f{s}", name=f"f{s}")
        nc.sync.dma_start(ft[:], Fd[row : row + rows, :])
        f_tiles.append(ft)
        row += rows
    f_combo = []
    for j in range(3):
        ft = fpool.tile([128, T + 2], BF16, tag=f"fc{j}", name=f"fc{j}")
        nc.sync.dma_start(ft[:], Fd[row : row + 128, :])
        f_combo.append(ft)
        row += 128
    # per-j chunk list: (f_tile, rows, col_off) in accumulation order
    chunk_rows = [128, 128, 128, 128, 64]
    w_tiles = {}
    row = 0
    for j in range(3):
        for ci, rows in enumerate(chunk_rows):
            wt = wpool.tile([rows, O], BF16, tag=f"w{j}_{ci}", name=f"w{j}_{ci}")
            nc.sync.dma_start(wt[:], Wd[row : row + rows, :])
            w_tiles[(j, ci)] = wt
            row += rows

    # --- sim row space: sq/pl (bf16) -> ones-matmul partition reduce -> n/dl rows ---
    n_sb = rowpool.tile([128, T + 2], BF16, tag="n_sb")
    dl_sb = rowpool.tile([128, T + 2], BF16, tag="dl_sb")
    for sb in (n_sb, dl_sb):
        nc.vector.memset(sb[:, 0:1], 0.0)
        nc.vector.memset(sb[:, T + 1 : T + 2], 0.0)

    # window-extraction variants; n_sb col 1+t = n[t]; dl_sb col 1+t = dl[t].
    variants = [
        ("nT0", n_sb, 1),
        ("nTm", n_sb, 0),
        ("nTp", n_sb, 2),
        ("dT0", dl_sb, 1),
        ("dTp", dl_sb, 2),
    ]
    with tc.tile_pool(name="redpsum", bufs=1, space="PSUM") as redpsum:
        xt_all = redpsum.tile([128, 5 * TQ], F32, tag="xt", name="xt_all")
        for b in range(NB // 2):
            s2b = redpsum.tile([128, 1024], F32, tag="s2b", name=f"s2b{b}")
            dlb = redpsum.tile([128, 1024], F32, tag="dlb", name=f"dlb{b}")
            for p in range(CP):
                esl = e_slices[(b, p)]
                sq = sqpool.tile([128, 1024], BF16, tag="sq", name=f"sq{b}_{p}")
                pl = sqpool.tile([128, 1024], BF16, tag="pl", name=f"pl{b}_{p}")
                nc.scalar.square(sq[:], esl[:, 1:1025])
                nc.vector.tensor_mul(pl[:], esl[:, 1:1025], esl[:, 0:1024])
                for h in range(2):  # matmul out limited to one PSUM bank (512 f32)
                    hs = slice(512 * h, 512 * h + 512)
                    nc.tensor.matmul(
                        s2b[:, hs], ones_t[:], sq[:, hs],
                        start=(p == 0), stop=(p == CP - 1),
                    )
                    nc.tensor.matmul(
                        dlb[:, hs], ones_t[:], pl[:, hs],
                        start=(p == 0), stop=(p == CP - 1),
                    )
            # evacuate: n = sqrt(s2) via ACT, dl plain copy via DVE (bf16 out)
            nc.scalar.sqrt(n_sb[:, 1 + 1024 * b : 1025 + 1024 * b], s2b[:])
            nc.vector.tensor_copy(dl_sb[:, 1 + 1024 * b : 1025 + 1024 * b], dlb[:])
        for v, (name, src, off) in enumerate(variants):
            for q in range(TQ):
                nc.tensor.matmul(
                    xt_all[:, 32 * v + q : 32 * v + q + 1],
                    src[:, off + 128 * q : off + 128 * q + 128],
                    e0[:],
                    start=True,
                    stop=True,
                )
        xt_sb = simpool.tile([128, 5 * TQ], F32, tag="xt_sb", name="xt_sb")
        nc.vector.tensor_copy(xt_sb[:], xt_all[:])
    cols = {
        name: xt_sb[:, 32 * v : 32 * v + 32]
        for v, (name, _, _) in enumerate(variants)
    }

    # sims: sim_l = dT0 / max(nT0*nTm, tiny); sim_r = dTp / max(nT0*nTp, tiny)
    sim_l = simpool.tile([128, TQ], F32, tag="sim_l")
    sim_r = simpool.tile([128, TQ], F32, tag="sim_r")
    for sim, nx, dx in ((sim_l, "nTm", "dT0"), (sim_r, "nTp", "dTp")):
        prod = simpool.tile([128, TQ], F32, tag=f"prod_{nx}", name=f"prod_{nx}")
        nc.vector.tensor_mul(prod[:], cols["nT0"], cols[nx])
        nc.vector.tensor_scalar_max(prod[:], prod[:], 1e-30)
        nc.vector.reciprocal(prod[:], prod[:])
        nc.vector.tensor_mul(sim[:], cols[dx], prod[:])

    # --- main conv: out_T[t,o] accumulated per 128-t tile, 15 chunks each ---
    per_j = [[], [], []]
    for j in range(3):
        for k in range(K):  # big chunks: shift = k
            per_j[j].append((f_tiles[_BIG[j]], 128, k, w_tiles[(j, k)]))
        per_j[j].append((f_combo[j], 128, 0, w_tiles[(j, 3)]))  # combo (k0|k1)
        per_j[j].append((f_tiles[_SMALL[j]], 64, 2, w_tiles[(j, 4)]))  # small k2

    convpsum = ctx.enter_context(tc.tile_pool(name="convpsum", bufs=2, space="PSUM"))
    for q in range(TQ):
        psums = [
            convpsum.tile([128, O], F32, tag=f"P{j}", name=f"P{j}_{q}")
            for j in range(3)
        ]
        for j in range(3):
            cl = per_j[j]
            for idx, (ft, rows, off, wt) in enumerate(cl):
                nc.tensor.matmul(
                    psums[j][:],
                    ft[0:rows, off + 128 * q : off + 128 * q + 128],
                    wt[0:rows, :],
                    start=(idx == 0),
                    stop=(idx == len(cl) - 1),
                )
        # epilogue (one PSUM read per instruction):
        #   ACT: tmp  = P0 * sim_l          (PSUM -> SBUF, per-partition scale)
        #   DVE: tmp2 = (P2 * sim_r) + tmp  (PSUM + SBUF)
        #   DVE: osb  = P1 + tmp2           (PSUM + SBUF)
        tmp = outpool.tile([128, O], F32, tag="tmp", name=f"tmp{q}")
        nc.scalar.mul(tmp[:], psums[0][:], sim_l[:, q : q + 1])
        tmp2 = outpool.tile([128, O], F32, tag="tmp2", name=f"tmp2_{q}")
        nc.vector.scalar_tensor_tensor(
            tmp2[:],
            psums[2][:],
            sim_r[:, q : q + 1],
            tmp[:],
            op0=mybir.AluOpType.mult,
            op1=mybir.AluOpType.add,
        )
        osb = outpool.tile([128, O], F32, tag="osb", name=f"osb{q}")
        nc.vector.tensor_add(osb[:], psums[1][:], tmp2[:])
        nc.sync.dma_start(Od[128 * q : 128 * q + 128, :], osb[:])


_NC_CACHE = {}


def _get_nc():
    if "nc" not in _NC_CACHE:
        _NC_CACHE["nc"] = build_kernel()
    return _NC_CACHE["nc"]


def kernel(feature, embedding, weight):
    in_maps = host_prep(feature, embedding, weight)
    nc = _get_nc()
    res = run_bass_kernel_spmd(nc, in_maps, core_ids=list(range(B)))
    out = np.stack([res.results[b]["out"].T for b in range(B)])  # [B, O, T]
    return np.ascontiguousarray(out)
